# revision 4
# baseline (speedup 1.0000x reference)
"""Multi-head self-attention Trainium2 kernel (8-core SPMD, no collectives).

Problem: B=4, S=2048, E=1024, H=16, D=64, fp32 I/O.

Sharding: data-parallel over (batch, seq-half): core c handles batch c//2,
query rows [half*1024, half*1024+1024). K/V for the full batch are computed
redundantly by the two cores sharing a batch (cheaper than a collective).

On-chip dataflow (per core), everything in "transposed" space so no on-device
transposes are needed (x is pre-transposed on the host):
  xT [e, s]  --matmul-->  QT [dq, s], KT [dk, s]  (proj outputs transposed)
  xT as lhsT --matmul-->  V  [s, hd]              (natural layout)
  scoresT[k, q] = (KT_h)^T-free matmul: lhsT=KT_h[d,kt], rhs=QT_h[d,q]
  expT = exp(scoresT) on ScalarE (PSUM -> SBUF bf16)
  outT_h[d, q] (+ sumexp row) = matmul(lhsT=V_aug_h[k, 65], rhs=expT[k, q])
  normalize via reciprocal + gpsimd partition-broadcast + DVE multiply
  out[s, e] = matmul(lhsT=attn_outT[hd, s], rhs=WO[hd, e]) + bO
Biases are folded in with K=1 ones-row matmuls (V, out) or per-partition
adds during PSUM eviction (Q, K). The ones column appended to V makes the
softmax denominator accumulate for free in PSUM partition 64 (M=65).
"""

import os
import sys

import numpy as np

for _p in ("/opt/trn_rl_repo", "/root/.axon_site/_ro/trn_rl_repo"):
    if os.path.isdir(_p) and _p not in sys.path:
        sys.path.append(_p)

import concourse.mybir as mybir
from concourse import bacc
from concourse.bass_utils import run_bass_kernel_spmd
from concourse.tile import TileContext

F16 = mybir.dt.float16
BF16 = mybir.dt.bfloat16
F32 = mybir.dt.float32
EXP = mybir.ActivationFunctionType.Exp

B, S, E = 4, 2048, 1024
H, D = 16, 64
HPAIRS = H // 2        # 8 head pairs (2 heads share a 128-partition block)
SQ = S // 2            # 1024 query rows per core
ET = E // 128          # 8 contraction tiles over embed dim
KTILES = S // 128      # 16 key tiles
DV_AUG = H * (D + 1)   # 1040: V columns with a ones column per head
N_CORES = 8

_CACHE: dict = {}


def _build():
    nc = bacc.Bacc("TRN2", target_bir_lowering=False)

    xt_d = nc.dram_tensor("xt", [ET, 128, S], F16, kind="ExternalInput")
    wq_d = nc.dram_tensor("wq", [ET, 128, E], F16, kind="ExternalInput")
    wk_d = nc.dram_tensor("wk", [ET, 128, E], F16, kind="ExternalInput")
    wv_d = nc.dram_tensor("wv", [ET, 128, DV_AUG], F16, kind="ExternalInput")
    wo_d = nc.dram_tensor("wo", [ET, 128, E], F16, kind="ExternalInput")
    bqk_d = nc.dram_tensor("bqk", [128, 2 * ET], F32, kind="ExternalInput")
    brow_d = nc.dram_tensor("brow", [1, DV_AUG + E], F16, kind="ExternalInput")
    out_d = nc.dram_tensor("out", [SQ, E], F32, kind="ExternalOutput")

    with nc.allow_low_precision("intentional fp16/bf16 activations"), TileContext(
        nc
    ) as tc:
        with (
            tc.tile_pool(name="persist", bufs=1) as persist,
            tc.tile_pool(name="qtkt", bufs=2) as qtkt,
            tc.tile_pool(name="work", bufs=2) as work,
            tc.tile_pool(name="pbig", bufs=2, space="PSUM") as pbig,
            tc.tile_pool(name="pav", bufs=1, space="PSUM") as pav,
        ):
            v_sb = persist.tile([128, KTILES, DV_AUG], BF16, name="v_sb")
            aout_sb = persist.tile([128, ET, SQ], F16, name="aout_sb")
            wo_sb = persist.tile([128, ET, E], F16, name="wo_sb")
            bqk_sb = persist.tile([128, 2 * ET], F32, name="bqk_sb")
            brow_sb = persist.tile([1, DV_AUG + E], F16, name="brow_sb")
            ones_sb = persist.tile([1, 128], F16, name="ones_sb")
            ones_bf = persist.tile([1, 128], BF16, name="ones_bf")
            nc.vector.memset(ones_sb, 1.0)
            nc.vector.memset(ones_bf, 1.0)
            nc.sync.dma_start(out=bqk_sb, in_=bqk_d[:, :])
            nc.sync.dma_start(out=brow_sb, in_=brow_d[:, :])

            def big(name):
                return pbig.tile([128, 1024], F32, tag="big", name=name)

            with tc.tile_pool(name="proj", bufs=1) as proj:
                xt_sb = proj.tile([128, ET, S], F16, name="xt_sb")
                wq_sb = proj.tile([128, ET, E], F16, name="wq_sb")
                wk_sb = proj.tile([128, ET, E], F16, name="wk_sb")
                wv_sb = proj.tile([128, ET, DV_AUG], F16, name="wv_sb")
                for et in range(ET):
                    nc.sync.dma_start(out=xt_sb[:, et, :], in_=xt_d[et, :, :])
                    nc.sync.dma_start(out=wv_sb[:, et, :], in_=wv_d[et, :, :])
                    nc.sync.dma_start(out=wq_sb[:, et, :], in_=wq_d[et, :, :])
                    nc.sync.dma_start(out=wk_sb[:, et, :], in_=wk_d[et, :, :])
                    nc.sync.dma_start(out=wo_sb[:, et, :], in_=wo_d[et, :, :])

                # ---- V projection: V_aug[s, j] = x @ WV_aug + bV_aug ----
                for st in range(KTILES):
                    pv = big(f"pv_{st}")
                    pv16 = big(f"pv16_{st}")
                    for et in range(ET):
                        lhs = xt_sb[:, et, st * 128 : (st + 1) * 128]
                        st0 = et == 0
                        nc.tensor.matmul(
                            pv[:, 0:512], lhsT=lhs, rhs=wv_sb[:, et, 0:512],
                            start=st0, stop=False,
                        )
                        nc.tensor.matmul(
                            pv[:, 512:1024], lhsT=lhs, rhs=wv_sb[:, et, 512:1024],
                            start=st0, stop=False,
                        )
                        nc.tensor.matmul(
                            pv16[:, 0:16], lhsT=lhs, rhs=wv_sb[:, et, 1024:1040],
                            start=st0, stop=False,
                        )
                    one = ones_sb[0:1, 0:128]
                    nc.tensor.matmul(
                        pv[:, 0:512], lhsT=one, rhs=brow_sb[0:1, 0:512],
                        start=False, stop=True,
                    )
                    nc.tensor.matmul(
                        pv[:, 512:1024], lhsT=one, rhs=brow_sb[0:1, 512:1024],
                        start=False, stop=True,
                    )
                    nc.tensor.matmul(
                        pv16[:, 0:16], lhsT=one, rhs=brow_sb[0:1, 1024:1040],
                        start=False, stop=True,
                    )
                    nc.vector.tensor_copy(out=v_sb[:, st, 0:1024], in_=pv)
                    nc.vector.tensor_copy(
                        out=v_sb[:, st, 1024:1040], in_=pv16[:, 0:16]
                    )

                # ---- per head-pair: QT/KT projection then attention ----
                for hp in range(HPAIRS):
                    qt_t = qtkt.tile([128, SQ], F16, tag="qt", name=f"qt_{hp}")
                    pq = big(f"pq_{hp}")
                    for q2 in range(2):
                        for et in range(ET):
                            nc.tensor.matmul(
                                pq[:, q2 * 512 : (q2 + 1) * 512],
                                lhsT=wq_sb[:, et, hp * 128 : (hp + 1) * 128],
                                rhs=xt_sb[:, et, q2 * 512 : (q2 + 1) * 512],
                                start=(et == 0), stop=(et == ET - 1),
                            )
                    nc.vector.tensor_scalar_add(
                        out=qt_t, in0=pq, scalar1=bqk_sb[:, hp : hp + 1]
                    )

                    kt_t = qtkt.tile([128, S], F16, tag="kt", name=f"kt_{hp}")
                    for kk in range(2):
                        pk = big(f"pk_{hp}_{kk}")
                        for q2 in range(2):
                            base = kk * 1024 + q2 * 512
                            for et in range(ET):
                                nc.tensor.matmul(
                                    pk[:, q2 * 512 : (q2 + 1) * 512],
                                    lhsT=wk_sb[:, et, hp * 128 : (hp + 1) * 128],
                                    rhs=xt_sb[:, et, base : base + 512],
                                    start=(et == 0), stop=(et == ET - 1),
                                )
                        nc.vector.tensor_scalar_add(
                            out=kt_t[:, kk * 1024 : (kk + 1) * 1024],
                            in0=pk,
                            scalar1=bqk_sb[:, ET + hp : ET + hp + 1],
                        )

                    # attention accumulators: [65, 512] per (head, q-chunk);
                    # partition 64 accumulates sumexp via the ones column.
                    av = {}
                    for h in range(2):
                        for q2 in range(2):
                            av[(h, q2)] = pav.tile(
                                [65, 512], F32, tag=f"av{h}{q2}",
                                name=f"av_{hp}_{h}_{q2}",
                            )
                    for t in range(KTILES):
                        for h in range(2):
                            hg = hp * 2 + h
                            sc = big(f"sc_{hp}_{t}_{h}")
                            for q2 in range(2):
                                nc.tensor.matmul(
                                    sc[:, q2 * 512 : (q2 + 1) * 512],
                                    lhsT=kt_t[
                                        h * 64 : (h + 1) * 64,
                                        t * 128 : (t + 1) * 128,
                                    ],
                                    rhs=qt_t[
                                        h * 64 : (h + 1) * 64,
                                        q2 * 512 : (q2 + 1) * 512,
                                    ],
                                    start=True, stop=True,
                                )
                            ex = work.tile(
                                [128, 1024], BF16, tag="ex", bufs=4,
                                name=f"ex_{hp}_{t}_{h}",
                            )
                            nc.scalar.activation(out=ex, in_=sc, func=EXP)
                            for q2 in range(2):
                                nc.tensor.matmul(
                                    av[(h, q2)],
                                    lhsT=v_sb[
                                        :, t, hg * (D + 1) : (hg + 1) * (D + 1)
                                    ],
                                    rhs=ex[:, q2 * 512 : (q2 + 1) * 512],
                                    start=(t == 0), stop=(t == KTILES - 1),
                                )

                    # normalize: out_h[d, q] * (1 / sumexp[q]) and park into
                    # attn-out at partition block hp (head B shifted via DMA).
                    # 1/sumexp lives on partition 64; broadcast across the 64
                    # head dims via row-shift DMA + K=1 ones matmul.
                    for h in range(2):
                        recip_t = work.tile(
                            [128, SQ], BF16, tag="recip", name=f"recip_{hp}_{h}"
                        )
                        rrow_t = work.tile(
                            [1, SQ], BF16, tag="rrow", name=f"rrow_{hp}_{h}"
                        )
                        rbc_sb = work.tile(
                            [64, SQ], F32, tag="rbc", name=f"rbc_{hp}_{h}"
                        )
                        tmp_t = work.tile(
                            [64, SQ], F16, tag="tmp", name=f"tmp_{hp}_{h}"
                        )
                        for q2 in range(2):
                            nc.vector.reciprocal(
                                out=recip_t[64:65, q2 * 512 : (q2 + 1) * 512],
                                in_=av[(h, q2)][64:65, :],
                            )
                        nc.sync.dma_start(
                            out=rrow_t[0:1, :], in_=recip_t[64:65, :]
                        )
                        rb_ps = big(f"rb_{hp}_{h}")
                        for q2 in range(2):
                            nc.tensor.matmul(
                                rb_ps[0:64, q2 * 512 : (q2 + 1) * 512],
                                lhsT=ones_bf[0:1, 0:64],
                                rhs=rrow_t[0:1, q2 * 512 : (q2 + 1) * 512],
                                start=True, stop=True,
                            )
                        nc.vector.tensor_copy(out=rbc_sb, in_=rb_ps[0:64, :])
                        for q2 in range(2):
                            nc.vector.tensor_mul(
                                out=tmp_t[:, q2 * 512 : (q2 + 1) * 512],
                                in0=av[(h, q2)][0:64, :],
                                in1=rbc_sb[:, q2 * 512 : (q2 + 1) * 512],
                            )
                        nc.sync.dma_start(
                            out=aout_sb[h * 64 : (h + 1) * 64, hp, :], in_=tmp_t
                        )

            # ---- output projection: out[s, e] = attn_out @ WO + bO ----
            for st in range(ET):
                po = big(f"po_{st}")
                for ec in range(2):
                    nc.tensor.matmul(
                        po[:, ec * 512 : (ec + 1) * 512],
                        lhsT=ones_sb[0:1, 0:128],
                        rhs=brow_sb[0:1, DV_AUG + ec * 512 : DV_AUG + (ec + 1) * 512],
                        start=True, stop=False,
                    )
                    for ht in range(ET):
                        nc.tensor.matmul(
                            po[:, ec * 512 : (ec + 1) * 512],
                            lhsT=aout_sb[:, ht, st * 128 : (st + 1) * 128],
                            rhs=wo_sb[:, ht, ec * 512 : (ec + 1) * 512],
                            start=False, stop=(ht == ET - 1),
                        )
                ot = work.tile([128, E], F32, tag="ot", name=f"ot_{st}")
                nc.vector.tensor_copy(out=ot, in_=po)
                nc.sync.dma_start(
                    out=out_d[st * 128 : (st + 1) * 128, :], in_=ot
                )

    nc.finalize()
    return nc


def _prep_inputs(x, WQ, bQ, WK, bK, WV, bV, WO, bO):
    f16 = np.float16
    x = np.asarray(x, np.float32)
    WQ = np.asarray(WQ, np.float32)
    WK = np.asarray(WK, np.float32)
    WV = np.asarray(WV, np.float32)
    WO = np.asarray(WO, np.float32)
    bQ = np.asarray(bQ, np.float32)
    bK = np.asarray(bK, np.float32)
    bV = np.asarray(bV, np.float32)
    bO = np.asarray(bO, np.float32)

    wq_np = np.ascontiguousarray(WQ.reshape(ET, 128, E)).astype(f16)
    wk_np = np.ascontiguousarray(WK.reshape(ET, 128, E)).astype(f16)
    wo_np = np.ascontiguousarray(WO.reshape(ET, 128, E)).astype(f16)
    wv_aug = np.zeros((E, H, D + 1), np.float32)
    wv_aug[:, :, :D] = WV.reshape(E, H, D)
    wv_np = wv_aug.reshape(ET, 128, DV_AUG).astype(f16)

    bqk_np = np.empty((128, 2 * ET), np.float32)
    bqk_np[:, :ET] = bQ.reshape(ET, 128).T
    bqk_np[:, ET:] = bK.reshape(ET, 128).T

    bv_aug = np.zeros((H, D + 1), np.float32)
    bv_aug[:, :D] = bV.reshape(H, D)
    bv_aug[:, D] = 1.0
    brow_np = np.concatenate([bv_aug.reshape(-1), bO]).reshape(1, -1).astype(f16)

    shared = {
        "wq": wq_np, "wk": wk_np, "wv": wv_np, "wo": wo_np,
        "bqk": bqk_np, "brow": brow_np,
    }
    in_maps = []
    for c in range(N_CORES):
        b, half = c // 2, c % 2
        xb = x[b]
        qrows = xb[half * SQ : (half + 1) * SQ]
        orows = xb[(1 - half) * SQ : (2 - half) * SQ]
        # this core's query columns first; attention is permutation-
        # invariant over key order so K/V consistency is preserved
        xt = np.concatenate([qrows.T, orows.T], axis=1)
        xt_np = np.ascontiguousarray(xt.reshape(ET, 128, S)).astype(f16)
        in_maps.append({"xt": xt_np, **shared})
    return in_maps


def kernel(x, WQ, bQ, WK, bK, WV, bV, WO, bO):
    if "nc" not in _CACHE:
        _CACHE["nc"] = _build()
    nc = _CACHE["nc"]
    in_maps = _prep_inputs(x, WQ, bQ, WK, bK, WV, bV, WO, bO)
    res = run_bass_kernel_spmd(nc, in_maps, core_ids=list(range(N_CORES)))
    _CACHE["last_result"] = res
    out = np.empty((B, S, E), np.float32)
    for c, r in enumerate(res.results):
        b, half = c // 2, c % 2
        out[b, half * SQ : (half + 1) * SQ] = r["out"]
    return out


# revision 5
# speedup vs baseline: 1.2393x; 1.2393x over previous
"""Multi-head self-attention Trainium2 kernel (8-core SPMD, no collectives).

Problem: B=4, S=2048, E=1024, H=16, D=64, fp32 I/O.

Sharding: data-parallel over (batch, seq-half): core c handles batch c//2,
query rows [half*1024, half*1024+1024). K/V for the full batch are computed
redundantly by the two cores sharing a batch (cheaper than a collective).

On-chip dataflow (per core), everything in "transposed" space so no on-device
transposes are needed (x is pre-transposed on the host):
  xT [e, s]  --matmul-->  QT [dq, s], KT [dk, s]  (proj outputs transposed)
  xT as lhsT --matmul-->  V  [s, hd]              (natural layout)
  scoresT[k, q]: lhsT=KT_h[d, ktile], rhs=QT_h[d, q]; the two heads of a
    pair sit in partition halves 0:64 / 64:128, so their score matmuls hit
    disjoint PE row groups and can overlap.
  expT = exp(scoresT) on ScalarE (PSUM -> SBUF bf16), one [128,1024] call
    covering both heads of the pair.
  outT_h[d, q] (+ sumexp row 64) = matmul(lhsT=V_aug_h[k, 65], rhs=expT)
    where V_aug has a constant ones column (memset, no matmul needed).
  normalize: sumexp rows are reshaped partition-major via a DRAM bounce so
    one cheap [128,16] reciprocal covers the whole pair, then broadcast
    across the 64 head dims with a K=1 ones matmul, multiply on VectorE.
  out[s, e] = matmul(lhsT=attn_outT[hd, s], rhs=WO[hd, e]) + bO
Q/K biases are added per-partition during PSUM eviction; V/O biases come in
as K=1 ones-row matmuls at the start of each accumulation group.
"""

import os
import sys

import numpy as np

for _p in ("/opt/trn_rl_repo", "/root/.axon_site/_ro/trn_rl_repo"):
    if os.path.isdir(_p) and _p not in sys.path:
        sys.path.append(_p)

import concourse.mybir as mybir
from concourse import bacc
from concourse.bass_utils import run_bass_kernel_spmd
from concourse.tile import TileContext

F16 = mybir.dt.float16
BF16 = mybir.dt.bfloat16
F32 = mybir.dt.float32
EXP = mybir.ActivationFunctionType.Exp

B, S, E = 4, 2048, 1024
H, D = 16, 64
HPAIRS = H // 2        # 8 head pairs (2 heads share a 128-partition block)
SQ = S // 2            # 1024 query rows per core
ET = E // 128          # 8 contraction tiles over embed dim
KTILES = S // 128      # 16 key tiles
N_CORES = 8

_CACHE: dict = {}


def _build():
    nc = bacc.Bacc("TRN2", target_bir_lowering=False)

    xt_d = nc.dram_tensor("xt", [ET, 128, S], F16, kind="ExternalInput")
    wq_d = nc.dram_tensor("wq", [ET, 128, E], F16, kind="ExternalInput")
    wk_d = nc.dram_tensor("wk", [ET, 128, E], F16, kind="ExternalInput")
    wv_d = nc.dram_tensor("wv", [ET, 128, E], F16, kind="ExternalInput")
    wo_d = nc.dram_tensor("wo", [ET, 128, E], F16, kind="ExternalInput")
    bqk_d = nc.dram_tensor("bqk", [128, 2 * ET], F32, kind="ExternalInput")
    brow_d = nc.dram_tensor("brow", [1, 2 * E], F16, kind="ExternalInput")
    out_d = nc.dram_tensor("out", [SQ, E], F32, kind="ExternalOutput")

    with nc.allow_low_precision("intentional fp16/bf16 activations"), TileContext(
        nc
    ) as tc:
        with (
            tc.tile_pool(name="persist", bufs=1) as persist,
            tc.tile_pool(name="qtkt", bufs=2) as qtkt,
            tc.tile_pool(name="work", bufs=2) as work,
            tc.tile_pool(name="dscr", bufs=2, space="DRAM") as dscr,
            tc.tile_pool(name="pbig", bufs=2, space="PSUM") as pbig,
            tc.tile_pool(name="pav", bufs=1, space="PSUM") as pav,
        ):
            # V with a ones column per head: [k%128, ktile, head, 65]
            v_sb = persist.tile([128, KTILES, H, D + 1], BF16, name="v_sb")
            aout_sb = persist.tile([128, ET, SQ], F16, name="aout_sb")
            wo_sb = persist.tile([128, ET, E], F16, name="wo_sb")
            bqk_sb = persist.tile([128, 2 * ET], F32, name="bqk_sb")
            brow_sb = persist.tile([1, 2 * E], F16, name="brow_sb")
            ones_sb = persist.tile([1, 128], F16, name="ones_sb")
            ones_bf = persist.tile([1, 128], BF16, name="ones_bf")
            nc.vector.memset(ones_sb, 1.0)
            nc.vector.memset(ones_bf, 1.0)
            for h in range(H):
                nc.vector.memset(v_sb[:, :, h, D], 1.0)
            nc.sync.dma_start(out=bqk_sb, in_=bqk_d[:, :])
            nc.sync.dma_start(out=brow_sb, in_=brow_d[:, :])

            def big(name):
                return pbig.tile([128, 1024], F32, tag="big", name=name)

            with tc.tile_pool(name="proj", bufs=1) as proj:
                xt_sb = proj.tile([128, ET, S], F16, name="xt_sb")
                wq_sb = proj.tile([128, ET, E], F16, name="wq_sb")
                wk_sb = proj.tile([128, ET, E], F16, name="wk_sb")
                wv_sb = proj.tile([128, ET, E], F16, name="wv_sb")
                for et in range(ET):
                    nc.sync.dma_start(out=xt_sb[:, et, :], in_=xt_d[et, :, :])
                    nc.sync.dma_start(out=wv_sb[:, et, :], in_=wv_d[et, :, :])
                    nc.sync.dma_start(out=wq_sb[:, et, :], in_=wq_d[et, :, :])
                    nc.sync.dma_start(out=wk_sb[:, et, :], in_=wk_d[et, :, :])
                    nc.sync.dma_start(out=wo_sb[:, et, :], in_=wo_d[et, :, :])

                # ---- V projection: V[s, hd] = x @ WV + bV ----
                for st in range(KTILES):
                    pv = big(f"pv_{st}")
                    for et in range(ET):
                        lhs = xt_sb[:, et, st * 128 : (st + 1) * 128]
                        for c in range(2):
                            nc.tensor.matmul(
                                pv[:, c * 512 : (c + 1) * 512],
                                lhsT=lhs,
                                rhs=wv_sb[:, et, c * 512 : (c + 1) * 512],
                                start=(et == 0), stop=False,
                            )
                    for c in range(2):
                        nc.tensor.matmul(
                            pv[:, c * 512 : (c + 1) * 512],
                            lhsT=ones_sb[0:1, 0:128],
                            rhs=brow_sb[0:1, c * 512 : (c + 1) * 512],
                            start=False, stop=True,
                        )
                    nc.vector.tensor_copy(
                        out=v_sb[:, st, :, 0:D],
                        in_=pv.rearrange("p (h d) -> p h d", h=H),
                    )

                def project_pair(hp):
                    """QT/KT projection for head pair hp; returns (qt, kt)."""
                    qt_t = qtkt.tile([128, SQ], F16, tag="qt", name=f"qt_{hp}")
                    kt_t = qtkt.tile([128, S], F16, tag="kt", name=f"kt_{hp}")
                    pq = big(f"pq_{hp}")
                    for et in range(ET):
                        for q2 in range(2):
                            nc.tensor.matmul(
                                pq[:, q2 * 512 : (q2 + 1) * 512],
                                lhsT=wq_sb[:, et, hp * 128 : (hp + 1) * 128],
                                rhs=xt_sb[:, et, q2 * 512 : (q2 + 1) * 512],
                                start=(et == 0), stop=(et == ET - 1),
                            )
                    for q2 in range(2):
                        nc.vector.tensor_scalar_add(
                            out=qt_t[:, q2 * 512 : (q2 + 1) * 512],
                            in0=pq[:, q2 * 512 : (q2 + 1) * 512],
                            scalar1=bqk_sb[:, hp : hp + 1],
                        )
                    for kk in range(2):
                        pk = big(f"pk_{hp}_{kk}")
                        for et in range(ET):
                            for q2 in range(2):
                                base = kk * 1024 + q2 * 512
                                nc.tensor.matmul(
                                    pk[:, q2 * 512 : (q2 + 1) * 512],
                                    lhsT=wk_sb[:, et, hp * 128 : (hp + 1) * 128],
                                    rhs=xt_sb[:, et, base : base + 512],
                                    start=(et == 0), stop=(et == ET - 1),
                                )
                        for q2 in range(2):
                            nc.vector.tensor_scalar_add(
                                out=kt_t[
                                    :, kk * 1024 + q2 * 512 : kk * 1024 + (q2 + 1) * 512
                                ],
                                in0=pk[:, q2 * 512 : (q2 + 1) * 512],
                                scalar1=bqk_sb[:, ET + hp : ET + hp + 1],
                            )
                    return qt_t, kt_t

                # ---- attention over head pairs ----
                cur = project_pair(0)
                for hp in range(HPAIRS):
                    qt_t, kt_t = cur
                    av = {}
                    for h in range(2):
                        for q2 in range(2):
                            av[(h, q2)] = pav.tile(
                                [65, 512], F32, tag=f"av{h}{q2}",
                                name=f"av_{hp}_{h}_{q2}",
                            )
                    exs = {}
                    for t in range(KTILES):
                        for q2 in range(2):
                            sc = big(f"sc_{hp}_{t}_{q2}")
                            for h in range(2):
                                nc.tensor.matmul(
                                    sc[:, h * 512 : (h + 1) * 512],
                                    lhsT=kt_t[
                                        h * 64 : (h + 1) * 64,
                                        t * 128 : (t + 1) * 128,
                                    ],
                                    rhs=qt_t[
                                        h * 64 : (h + 1) * 64,
                                        q2 * 512 : (q2 + 1) * 512,
                                    ],
                                    start=True, stop=True,
                                )
                            ex = work.tile(
                                [128, 1024], BF16, tag="ex", bufs=4,
                                name=f"ex_{hp}_{t}_{q2}",
                            )
                            nc.scalar.activation(out=ex, in_=sc, func=EXP)
                            exs[q2] = ex
                        for h in range(2):
                            hg = hp * 2 + h
                            for q2 in range(2):
                                nc.tensor.matmul(
                                    av[(h, q2)],
                                    lhsT=v_sb[:, t, hg, :],
                                    rhs=exs[q2][:, h * 512 : (h + 1) * 512],
                                    start=(t == 0), stop=(t == KTILES - 1),
                                )

                    if hp + 1 < HPAIRS:
                        cur = project_pair(hp + 1)

                    # ---- normalize + park into attn-out ----
                    # gather the 4 sumexp rows (PSUM partition 64) into one
                    # [1, 2048] row, bounce through DRAM into a partition-
                    # major [128, 16] tile for one cheap reciprocal, bounce
                    # back, then K=1 ones-matmul broadcast per head.
                    stage = work.tile(
                        [65, 2048], BF16, tag="stage", bufs=1,
                        name=f"stage_{hp}",
                    )
                    for h in range(2):
                        for q2 in range(2):
                            c = h * 2 + q2
                            nc.vector.tensor_copy(
                                out=stage[64:65, c * 512 : (c + 1) * 512],
                                in_=av[(h, q2)][64:65, :],
                            )
                    scr1 = dscr.tile([2048], BF16, tag="scr1", name=f"scr1_{hp}")
                    scr2 = dscr.tile([2048], BF16, tag="scr2", name=f"scr2_{hp}")
                    rs_t = work.tile([128, 16], BF16, tag="rs", name=f"rs_{hp}")
                    rr_t = work.tile([128, 16], BF16, tag="rr", name=f"rr_{hp}")
                    rrow2 = work.tile(
                        [1, 2048], BF16, tag="rrow2", name=f"rrow2_{hp}"
                    )
                    nc.sync.dma_start(out=scr1[:], in_=stage[64:65, :])
                    nc.sync.dma_start(
                        out=rs_t[:, :], in_=scr1.rearrange("(a b) -> a b", a=128)
                    )
                    nc.vector.reciprocal(out=rr_t, in_=rs_t)
                    nc.sync.dma_start(out=scr2[:], in_=rr_t)
                    nc.sync.dma_start(
                        out=rrow2[0:1, :],
                        in_=scr2.rearrange("(a b) -> a b", a=1),
                    )
                    for h in range(2):
                        rb = big(f"rb_{hp}_{h}")
                        for q2 in range(2):
                            nc.tensor.matmul(
                                rb[0:64, q2 * 512 : (q2 + 1) * 512],
                                lhsT=ones_bf[0:1, 0:64],
                                rhs=rrow2[
                                    0:1, h * 1024 + q2 * 512 : h * 1024 + (q2 + 1) * 512
                                ],
                                start=True, stop=True,
                            )
                        rbc_sb = work.tile(
                            [64, SQ], F32, tag="rbc", name=f"rbc_{hp}_{h}"
                        )
                        nc.vector.tensor_copy(out=rbc_sb, in_=rb[0:64, :])
                        tmp_t = work.tile(
                            [64, SQ], F16, tag="tmp", name=f"tmp_{hp}_{h}"
                        )
                        for q2 in range(2):
                            nc.vector.tensor_mul(
                                out=tmp_t[:, q2 * 512 : (q2 + 1) * 512],
                                in0=av[(h, q2)][0:64, :],
                                in1=rbc_sb[:, q2 * 512 : (q2 + 1) * 512],
                            )
                        nc.sync.dma_start(
                            out=aout_sb[h * 64 : (h + 1) * 64, hp, :], in_=tmp_t
                        )

            # ---- output projection: out[s, e] = attn_out @ WO + bO ----
            for st in range(ET):
                po = big(f"po_{st}")
                for ec in range(2):
                    nc.tensor.matmul(
                        po[:, ec * 512 : (ec + 1) * 512],
                        lhsT=ones_sb[0:1, 0:128],
                        rhs=brow_sb[0:1, E + ec * 512 : E + (ec + 1) * 512],
                        start=True, stop=False,
                    )
                for ht in range(ET):
                    for ec in range(2):
                        nc.tensor.matmul(
                            po[:, ec * 512 : (ec + 1) * 512],
                            lhsT=aout_sb[:, ht, st * 128 : (st + 1) * 128],
                            rhs=wo_sb[:, ht, ec * 512 : (ec + 1) * 512],
                            start=False, stop=(ht == ET - 1),
                        )
                for ec in range(2):
                    ot = work.tile([128, 512], F32, tag="ot", name=f"ot_{st}_{ec}")
                    nc.vector.tensor_copy(
                        out=ot, in_=po[:, ec * 512 : (ec + 1) * 512]
                    )
                    nc.sync.dma_start(
                        out=out_d[
                            st * 128 : (st + 1) * 128, ec * 512 : (ec + 1) * 512
                        ],
                        in_=ot,
                    )

    nc.finalize()
    return nc


def _prep_inputs(x, WQ, bQ, WK, bK, WV, bV, WO, bO):
    f16 = np.float16
    x = np.asarray(x, np.float32)
    WQ = np.asarray(WQ, np.float32)
    WK = np.asarray(WK, np.float32)
    WV = np.asarray(WV, np.float32)
    WO = np.asarray(WO, np.float32)
    bQ = np.asarray(bQ, np.float32)
    bK = np.asarray(bK, np.float32)
    bV = np.asarray(bV, np.float32)
    bO = np.asarray(bO, np.float32)

    wq_np = np.ascontiguousarray(WQ.reshape(ET, 128, E)).astype(f16)
    wk_np = np.ascontiguousarray(WK.reshape(ET, 128, E)).astype(f16)
    wv_np = np.ascontiguousarray(WV.reshape(ET, 128, E)).astype(f16)
    wo_np = np.ascontiguousarray(WO.reshape(ET, 128, E)).astype(f16)

    bqk_np = np.empty((128, 2 * ET), np.float32)
    bqk_np[:, :ET] = bQ.reshape(ET, 128).T
    bqk_np[:, ET:] = bK.reshape(ET, 128).T

    brow_np = np.concatenate([bV, bO]).reshape(1, -1).astype(f16)

    shared = {
        "wq": wq_np, "wk": wk_np, "wv": wv_np, "wo": wo_np,
        "bqk": bqk_np, "brow": brow_np,
    }
    in_maps = []
    for c in range(N_CORES):
        b, half = c // 2, c % 2
        xb = x[b]
        qrows = xb[half * SQ : (half + 1) * SQ]
        orows = xb[(1 - half) * SQ : (2 - half) * SQ]
        # this core's query columns first; attention is permutation-
        # invariant over key order so K/V consistency is preserved
        xt = np.concatenate([qrows.T, orows.T], axis=1)
        xt_np = np.ascontiguousarray(xt.reshape(ET, 128, S)).astype(f16)
        in_maps.append({"xt": xt_np, **shared})
    return in_maps


def kernel(x, WQ, bQ, WK, bK, WV, bV, WO, bO):
    if "nc" not in _CACHE:
        _CACHE["nc"] = _build()
    nc = _CACHE["nc"]
    in_maps = _prep_inputs(x, WQ, bQ, WK, bK, WV, bV, WO, bO)
    res = run_bass_kernel_spmd(nc, in_maps, core_ids=list(range(N_CORES)))
    _CACHE["last_result"] = res
    out = np.empty((B, S, E), np.float32)
    for c, r in enumerate(res.results):
        b, half = c // 2, c % 2
        out[b, half * SQ : (half + 1) * SQ] = r["out"]
    return out
